# revision 15
# baseline (speedup 1.0000x reference)
"""BatchRGATLayer Trainium2 kernel (8 NeuronCores, data-parallel over (batch, row-half)).

kernel(**inputs) takes FULL inputs (x, edge, adj, W, W1, a), shards across 8
cores (core c -> batch c//2, rows (c%2)*256 .. +256), runs one SPMD Bass
program on all 8 cores, gathers to the full (4, 512, 256) output.

For row-half cores (c%2==1) the node axis is rolled by -256 on the host for
x, edge(j), adj(j) so the single SPMD program can treat local rows as [0,256).
Softmax and att@h are invariant to a consistent j-permutation.
"""

import sys

sys.path.insert(0, "/opt/trn_rl_repo")

from contextlib import ExitStack

import numpy as np

import concourse.bass as bass
import concourse.tile as tile
from concourse import bacc, mybir
from concourse.bass_utils import run_bass_kernel_spmd
from concourse.masks import make_identity

F32 = mybir.dt.float32
BF16 = mybir.dt.bfloat16
F16 = mybir.dt.float16
AF = mybir.ActivationFunctionType
ALU = mybir.AluOpType

# problem dims (hardcoded per spec)
B, N, IN_F, E_F, OUT_F = 4, 512, 256, 64, 256
R = 256          # rows per core
N_CORES = 8
ALPHA = 0.2

# tunables
JSPLIT = 0       # j < JSPLIT -> PE transpose+matmul route; j >= JSPLIT -> DVE route
JB = 128         # j-block per edge DMA tile
EDGE_BF16 = True # cast edge fp32->f16 during DMA (SWDGE); DVE mult runs 2x
TREE_REDUCE = True
ED_BUFS = 4
PROD_BUFS = 2
TT_BUFS = 3
DEBUG_NJB = None   # limit edge j-blocks processed per i-tile (debug only)
DEBUG_NIT = None   # limit i-tiles (debug only)
DEBUG_STAGE = None # emit only stages <= k (debug only); None = all

_CACHE = {}


def build_program():
    stage = 9 if DEBUG_STAGE is None else DEBUG_STAGE
    nc = bacc.Bacc("TRN2", target_bir_lowering=False, debug=False)

    edge_d = nc.dram_tensor("edge_s", [R, N, E_F], F32, kind="ExternalInput").ap()
    adj_d = nc.dram_tensor("adj_s", [R, N], F32, kind="ExternalInput").ap()
    x_d = nc.dram_tensor("x_b", [N, IN_F], F32, kind="ExternalInput").ap()
    w_d = nc.dram_tensor("W", [IN_F, OUT_F], F32, kind="ExternalInput").ap()
    w1_d = nc.dram_tensor("W1", [E_F, OUT_F], F32, kind="ExternalInput").ap()
    a_d = nc.dram_tensor("a", [3 * OUT_F, 1], F32, kind="ExternalInput").ap()
    out_d = nc.dram_tensor("out_s", [R, OUT_F], F32, kind="ExternalOutput").ap()

    NIT = R // 128            # i-tiles per core (2)
    NJT = N // 128            # j-tiles (4)
    NFT = IN_F // 128         # f-tiles of W (2)
    NOT_ = OUT_F // 128       # o-tiles (2)
    NJB = N // JB             # edge DMA tiles per i-tile (16)
    if DEBUG_NJB is not None:
        NJB = DEBUG_NJB
    PAIRS_PER_TILE = JB // 2
    NIT_RUN = NIT if DEBUG_NIT is None else DEBUG_NIT

    ctx = ExitStack()
    with tile.TileContext(nc) as tc, ctx:
        consts = ctx.enter_context(tc.tile_pool(name="consts", bufs=1))
        sb1 = ctx.enter_context(tc.tile_pool(name="sb1", bufs=1))

        def dbg_finish(src_ap, rows, cols):
            for it_ in range(NIT):
                o = sb1.tile([128, OUT_F], F32, tag=f"dbgout{it_}", name=f"dbgout{it_}")
                nc.gpsimd.memset(o[:], 0.0)
                nc.vector.tensor_copy(o[0:rows, 0:cols], src_ap)
                nc.sync.dma_start(out_d[bass.ts(it_, 128), :], o[:])

        ident = consts.tile([128, 128], F32)
        make_identity(nc, ident)
        ident16 = consts.tile([128, 128], F16)
        make_identity(nc, ident16)
        ones_row = consts.tile([1, 128], F32)
        nc.gpsimd.memset(ones_row[:], 1.0)

        # ---- persistent SBUF tensors (coalesced loads) ----
        w_all = consts.tile([128, NFT * OUT_F], F32)
        w_sb = [w_all[:, bass.ts(ft, OUT_F)] for ft in range(NFT)]
        w1_sb = consts.tile([E_F, OUT_F], F32)
        a_all = consts.tile([1, 3 * OUT_F], F32)
        a1_row = a_all[:, 0:OUT_F]
        a2_row = a_all[:, OUT_F : 2 * OUT_F]
        a3_row = a_all[:, 2 * OUT_F : 3 * OUT_F]
        acol6 = consts.tile([128, 6], F32)
        a2_col = [acol6[:, 2 + ot : 3 + ot] for ot in range(NOT_)]
        x_all = sb1.tile([128, NJT * IN_F], F32)
        x_sb = [x_all[:, bass.ts(rt, IN_F)] for rt in range(NJT)]
        adj_all = sb1.tile([128, NIT * N], F32)
        adj_sb = [adj_all[:, bass.ts(it, N)] for it in range(NIT)]
        xT_sb = [sb1.tile([128, N], F32, tag=f"xT_sb{ft}", name=f"xT_sb{ft}") for ft in range(NFT)]
        h_sb = [sb1.tile([128, OUT_F], F32, tag=f"h_sb{rt}", name=f"h_sb{rt}") for rt in range(NJT)]
        h16_sb = [sb1.tile([128, OUT_F], F16, tag=f"h16_sb{rt}", name=f"h16_sb{rt}") for rt in range(NJT)]
        hT_sb = [sb1.tile([128, N], F32, tag=f"hT_sb{ot}", name=f"hT_sb{ot}") for ot in range(NOT_)]
        mask_sb = [sb1.tile([128, N], F16, tag=f"mask_sb{it}", name=f"mask_sb{it}") for it in range(NIT)]
        si_col = [sb1.tile([128, 1], F32, tag=f"si_col{it}", name=f"si_col{it}") for it in range(NIT)]
        sj_rep = sb1.tile([128, N], F32)
        expbias = consts.tile([128, 1], F32)
        EDT = F16 if EDGE_BF16 else F32
        wcol = consts.tile([128, E_F], EDT)
        w2 = consts.tile([128, 2], F32)
        w1a3 = consts.tile([E_F, 1], F32)
        w1a3_row = consts.tile([1, E_F], F32)

        # ---- input DMAs (one per dram tensor; a gets two tiny views) ----
        nc.sync.dma_start(a_all[:], a_d[:, :].rearrange("a b -> b a"))
        nc.sync.dma_start(
            acol6[:].rearrange("p (c b) -> p c b", b=1),
            a_d[:, :].rearrange("(c p) b -> p c b", p=128),
        )
        nc.sync.dma_start(w1_sb[:], w1_d[:, :])
        nc.sync.dma_start(
            w_all[:].rearrange("p (ft f) -> p ft f", f=OUT_F),
            w_d[:, :].rearrange("(ft p) f -> p ft f", p=128),
        )
        nc.sync.dma_start(
            x_all[:].rearrange("p (rt f) -> p rt f", f=IN_F),
            x_d[:, :].rearrange("(rt p) f -> p rt f", p=128),
        )
        nc.sync.dma_start(
            adj_all[:].rearrange("p (it j) -> p it j", j=N),
            adj_d[:, :].rearrange("(it p) j -> p it j", p=128),
        )

        nc.gpsimd.memset(expbias[:], -11.0)

        if stage == 1:
            dbg_finish(adj_sb[0][:, 0:OUT_F], 128, OUT_F)

        # ---- setup stage (scoped PSUM pool) ----
        if stage >= 2:
          with tc.tile_pool(name="ps_setup", bufs=2, space="PSUM") as psx:
            # w1a3 = W1 @ a3
            a3_rep_ps = psx.tile([E_F, OUT_F], F32, tag="mps")
            nc.tensor.matmul(a3_rep_ps[:], ones_row[:, 0:E_F], a3_row[:])
            a3_rep = sb1.tile([E_F, OUT_F], F32)
            nc.scalar.copy(a3_rep[:], a3_rep_ps[:])
            ttr_scratch = sb1.tile([E_F, OUT_F], F32)
            nc.vector.tensor_tensor(ttr_scratch[:], w1_sb[:], a3_rep[:], ALU.mult)
            nc.vector.reduce_sum(w1a3[:], ttr_scratch[:], axis=mybir.AxisListType.X)
            w1a3_row_ps = psx.tile([1, E_F], F32, tag="mps")
            nc.tensor.transpose(w1a3_row_ps[:], w1a3[:], ident[0:E_F, 0:E_F])
            nc.scalar.copy(w1a3_row[:], w1a3_row_ps[:])
            wcol_ps = psx.tile([128, E_F], F32, tag="mps")
            nc.tensor.matmul(wcol_ps[:], ones_row[:], w1a3_row[:])
            nc.scalar.copy(wcol[:], wcol_ps[:])
            # w2 [128, 2] block-diag of w1a3
            nc.gpsimd.memset(w2[:], 0.0)
            nc.vector.tensor_copy(w2[0:E_F, 0:1], w1a3[:])
            nc.sync.dma_start(w2[E_F:128, 1:2], w1a3[:])

            if stage == 2:
                dbg_finish(wcol[:], 128, E_F)

            # xT
            if stage >= 3:
                for rt in range(NJT):
                    xt_ps = psx.tile([128, NFT * 128], F32, tag="mps")
                    for ft in range(NFT):
                        nc.tensor.transpose(
                            xt_ps[:, bass.ts(ft, 128)],
                            x_sb[rt][:, bass.ts(ft, 128)],
                            ident[:],
                        )
                    for ft in range(NFT):
                        nc.scalar.copy(
                            xT_sb[ft][:, bass.ts(rt, 128)], xt_ps[:, bass.ts(ft, 128)]
                        )
                if stage == 3:
                    dbg_finish(xT_sb[0][:, 0:OUT_F], 128, OUT_F)

            # h = x @ W
            if stage >= 4:
                for rt in range(NJT):
                    h_ps = psx.tile([128, OUT_F], F32, tag="mps")
                    for ft in range(NFT):
                        nc.tensor.matmul(
                            h_ps[:], xT_sb[ft][:, bass.ts(rt, 128)], w_sb[ft][:],
                            start=(ft == 0), stop=(ft == NFT - 1),
                        )
                    nc.scalar.copy(h_sb[rt][:], h_ps[:])
                for rt in range(NJT):
                    nc.vector.tensor_copy(h16_sb[rt][:], h_sb[rt][:])
                if stage == 4:
                    dbg_finish(h_sb[0][:], 128, OUT_F)

            # hT = W^T x^T ; s_i ; s_j ; masks
            if stage >= 5:
                for ot in range(NOT_):
                    for rt in range(NJT):
                        ht_ps = psx.tile([128, 128], F32, tag="mps")
                        for ft in range(NFT):
                            nc.tensor.matmul(
                                ht_ps[:],
                                w_sb[ft][:, bass.ts(ot, 128)],
                                xT_sb[ft][:, bass.ts(rt, 128)],
                                start=(ft == 0), stop=(ft == NFT - 1),
                            )
                        nc.scalar.copy(hT_sb[ot][:, bass.ts(rt, 128)], ht_ps[:])

                a1rep_ps = psx.tile([128, OUT_F], F32, tag="mps")
                nc.tensor.matmul(a1rep_ps[:], ones_row[:], a1_row[:])
                a1rep = sb1.tile([128, OUT_F], F32)
                nc.scalar.copy(a1rep[:], a1rep_ps[:])
                for it in range(NIT):
                    scratch = sb1.tile([128, OUT_F], F32, tag="si_scratch")
                    nc.vector.tensor_tensor(scratch[:], h_sb[it][:], a1rep[:], ALU.mult)
                    nc.vector.reduce_sum(
                        si_col[it][:], scratch[:], axis=mybir.AxisListType.X
                    )

                sj_ps = psx.tile([1, N], F32, tag="mps")
                for ot in range(NOT_):
                    nc.tensor.matmul(
                        sj_ps[:], a2_col[ot][:], hT_sb[ot][:],
                        start=(ot == 0), stop=(ot == NOT_ - 1),
                    )
                sj_row = sb1.tile([1, N], F32)
                nc.scalar.copy(sj_row[:], sj_ps[:])
                sjrep_ps = psx.tile([128, N], F32, tag="mps")
                nc.tensor.matmul(sjrep_ps[:], ones_row[:], sj_row[:])
                nc.scalar.copy(sj_rep[:], sjrep_ps[:])

                for it in range(NIT):
                    nc.vector.tensor_scalar(
                        mask_sb[it][:], adj_sb[it][:], 0.0, None, op0=ALU.is_gt
                    )
                if stage == 5:
                    dbg_finish(sj_rep[:, 0:OUT_F], 128, OUT_F)

        # ---- main per-i-tile pipeline ----
        if stage >= 6:
            ed_pool = ctx.enter_context(tc.tile_pool(name="ed", bufs=ED_BUFS))
            prod_pool = ctx.enter_context(tc.tile_pool(name="prod", bufs=PROD_BUFS))
            tT_ps_pool = ctx.enter_context(
                tc.tile_pool(name="tT_ps", bufs=TT_BUFS, space="PSUM")
            )
            tT_sb_pool = ctx.enter_context(tc.tile_pool(name="tT_sb", bufs=TT_BUFS))
            se_ps_pool = ctx.enter_context(tc.tile_pool(name="se_ps", bufs=2, space="PSUM"))
            soft_pool = ctx.enter_context(tc.tile_pool(name="soft", bufs=2))
            att_ps_pool = ctx.enter_context(tc.tile_pool(name="att_ps", bufs=2, space="PSUM"))
            attT_pool = ctx.enter_context(tc.tile_pool(name="attT", bufs=2))
            hp_ps_pool = ctx.enter_context(tc.tile_pool(name="hp_ps", bufs=2, space="PSUM"))
            out_pool = ctx.enter_context(tc.tile_pool(name="outp", bufs=2))

            for it in range(NIT_RUN):
                se_ps = (
                    se_ps_pool.tile([128, N], F32) if JSPLIT > 0 else None
                )  # PE-route s_e accumulator
                se_dve = soft_pool.tile([128, N], F32, tag="se_dve")

                for jb in range(NJB):
                    ed = ed_pool.tile([128, JB * E_F], EDT)
                    src_ap = edge_d[bass.ts(it, 128), bass.ts(jb, JB), :].rearrange(
                        "p a b -> p (a b)"
                    )
                    if EDGE_BF16:
                        nc.gpsimd.dma_start(ed[:], src_ap)
                    else:
                        nc.sync.dma_start(ed[:], src_ap)
                    if jb * JB < JSPLIT:
                        # PE route: transpose chunks then matmul with w2
                        q = 0
                        while q < PAIRS_PER_TILE:
                            qn = min(4, PAIRS_PER_TILE - q)
                            tps = tT_ps_pool.tile([128, 512], F32)
                            for k in range(qn):
                                nc.tensor.transpose(
                                    tps[:, bass.ts(k, 128)],
                                    ed[:, bass.ts(q + k, 128)],
                                    ident[:],
                                )
                            tsb = tT_sb_pool.tile([128, 512], F32)
                            nc.scalar.copy(tsb[:, 0 : qn * 128], tps[:, 0 : qn * 128])
                            for k in range(qn):
                                j0 = jb * JB + 2 * (q + k)
                                nc.tensor.matmul(
                                    se_ps[:, j0 : j0 + 2],
                                    tsb[:, bass.ts(k, 128)],
                                    w2[:],
                                )
                            q += qn
                    else:
                        # DVE route: broadcast-multiply + segmented (tree) reduce
                        prod = prod_pool.tile([128, JB * E_F], EDT)
                        nc.vector.tensor_tensor(
                            prod[:].rearrange("p (a b) -> p a b", b=E_F),
                            ed[:].rearrange("p (a b) -> p a b", b=E_F),
                            wcol[:, None, :].broadcast_to([128, JB, E_F]),
                            ALU.mult,
                        )
                        if TREE_REDUCE and EDGE_BF16:
                            v0 = prod[:].rearrange("p (a b) -> p a b", b=E_F)
                            t1 = prod_pool.tile([128, JB * 32], EDT, tag="t1")
                            nc.vector.tensor_tensor(
                                t1[:].rearrange("p (a b) -> p a b", b=32),
                                v0[:, :, 0:32], v0[:, :, 32:64], ALU.add,
                            )
                            v1 = t1[:].rearrange("p (a b) -> p a b", b=32)
                            t2 = prod_pool.tile([128, JB * 16], EDT, tag="t2")
                            nc.vector.tensor_tensor(
                                t2[:].rearrange("p (a b) -> p a b", b=16),
                                v1[:, :, 0:16], v1[:, :, 16:32], ALU.add,
                            )
                            v2 = t2[:].rearrange("p (a b) -> p a b", b=16)
                            t3 = prod_pool.tile([128, JB * 8], EDT, tag="t3")
                            nc.vector.tensor_tensor(
                                t3[:].rearrange("p (a b) -> p a b", b=8),
                                v2[:, :, 0:8], v2[:, :, 8:16], ALU.add,
                            )
                            nc.vector.reduce_sum(
                                se_dve[:, bass.ts(jb, JB)],
                                t3[:].rearrange("p (a b) -> p a b", b=8),
                                axis=mybir.AxisListType.X,
                            )
                        else:
                            nc.vector.reduce_sum(
                                se_dve[:, bass.ts(jb, JB)],
                                prod[:].rearrange("p (a b) -> p a b", b=E_F),
                                axis=mybir.AxisListType.X,
                            )

                # z = s_e + s_i + s_j ; leaky relu ; exp ; mask
                z = soft_pool.tile([128, N], F32, tag="z")
                jsp = min(JSPLIT, NJB * JB)
                jdve_hi = NJB * JB
                if jsp > 0:
                    nc.vector.scalar_tensor_tensor(
                        out=z[:, 0:jsp], in0=se_ps[:, 0:jsp], scalar=si_col[it][:],
                        in1=sj_rep[:, 0:jsp], op0=ALU.add, op1=ALU.add,
                    )
                if jsp < N:
                    src = se_dve[:, jsp:N] if jdve_hi > jsp else sj_rep[:, jsp:N]
                    nc.vector.scalar_tensor_tensor(
                        out=z[:, jsp:N], in0=src, scalar=si_col[it][:],
                        in1=sj_rep[:, jsp:N], op0=ALU.add, op1=ALU.add,
                    )
                zl = soft_pool.tile([128, N], F32, tag="zl")
                nc.vector.scalar_tensor_tensor(
                    out=zl[:], in0=z[:], scalar=ALPHA, in1=z[:],
                    op0=ALU.mult, op1=ALU.max,
                )
                p = soft_pool.tile([128, N], F16, tag="p")
                # bias keeps exp in fp16 range (softmax-invariant shift)
                nc.scalar.activation(p[:], zl[:], AF.Exp, bias=expbias[:])
                pm = soft_pool.tile([128, N], F16, tag="pm")
                nc.vector.tensor_tensor(pm[:], p[:], mask_sb[it][:], ALU.mult)

                denom = soft_pool.tile([128, 1], F32, tag="denom")
                nc.vector.reduce_sum(denom[:], pm[:], axis=mybir.AxisListType.X)
                rden = soft_pool.tile([128, 1], F32, tag="rden")
                nc.vector.reciprocal(rden[:], denom[:])

                # attT per j-tile, then h' = att @ h (accumulate over j-tiles)
                hp_ps = hp_ps_pool.tile([128, OUT_F], F32)
                for jt in range(NJT):
                    aps = att_ps_pool.tile([128, 128], F16)
                    nc.tensor.transpose(aps[:], pm[:, bass.ts(jt, 128)], ident16[:])
                    asb = attT_pool.tile([128, 128], F16)
                    nc.scalar.copy(asb[:], aps[:])
                    nc.tensor.matmul(
                        hp_ps[:], asb[:], h16_sb[jt][:],
                        start=(jt == 0), stop=(jt == NJT - 1),
                    )

                # normalize + ELU: out = max(exp(min(h'*r, 0)) - 1, h'*r)
                xx = out_pool.tile([128, OUT_F], F32, tag="xx")
                nc.scalar.mul(xx[:], hp_ps[:], rden[:])
                tmin = out_pool.tile([128, OUT_F], F32, tag="tmin")
                nc.vector.tensor_scalar(tmin[:], xx[:], 0.0, None, op0=ALU.min)
                ex = out_pool.tile([128, OUT_F], F32, tag="ex")
                nc.scalar.activation(ex[:], tmin[:], AF.Exp)
                ot_sb = out_pool.tile([128, OUT_F], F32, tag="ot_sb")
                nc.vector.scalar_tensor_tensor(
                    out=ot_sb[:], in0=ex[:], scalar=-1.0, in1=xx[:],
                    op0=ALU.add, op1=ALU.max,
                )
                nc.sync.dma_start(out_d[bass.ts(it, 128), :], ot_sb[:])

            for it_ in range(NIT_RUN, NIT):
                o = sb1.tile([128, OUT_F], F32, tag=f"padout{it_}", name=f"padout{it_}")
                nc.gpsimd.memset(o[:], 0.0)
                nc.sync.dma_start(out_d[bass.ts(it_, 128), :], o[:])

    nc.compile()
    return nc


def _shard(x, edge, adj, W, W1, a):
    in_maps = []
    for c in range(N_CORES):
        bi, half = c // 2, c % 2
        r0 = half * R
        if r0:
            xb = np.roll(x[bi], -r0, axis=0)
            ed = np.roll(edge[bi, r0 : r0 + R], -r0, axis=1)
            ad = np.roll(adj[bi, r0 : r0 + R], -r0, axis=1)
        else:
            xb = x[bi]
            ed = edge[bi, 0:R]
            ad = adj[bi, 0:R]
        in_maps.append(
            {
                "edge_s": np.ascontiguousarray(ed),
                "adj_s": np.ascontiguousarray(ad),
                "x_b": np.ascontiguousarray(xb),
                "W": W,
                "W1": W1,
                "a": a,
            }
        )
    return in_maps


def kernel(x, edge, adj, W, W1, a, _trace=False):
    if "nc" not in _CACHE:
        _CACHE["nc"] = build_program()
    nc = _CACHE["nc"]

    x = np.asarray(x, dtype=np.float32)
    edge = np.asarray(edge, dtype=np.float32)
    adj = np.asarray(adj, dtype=np.float32)
    W = np.ascontiguousarray(np.asarray(W, dtype=np.float32))
    W1 = np.ascontiguousarray(np.asarray(W1, dtype=np.float32))
    a = np.ascontiguousarray(np.asarray(a, dtype=np.float32).reshape(3 * OUT_F, 1))

    in_maps = _shard(x, edge, adj, W, W1, a)
    res = run_bass_kernel_spmd(
        nc, in_maps, core_ids=list(range(N_CORES)), trace=_trace
    )
    out = np.empty((B, N, OUT_F), dtype=np.float32)
    for c in range(N_CORES):
        bi, half = c // 2, c % 2
        out[bi, half * R : (half + 1) * R] = res.results[c]["out_s"]
    if _trace:
        _CACHE["last_exec_time_ns"] = res.exec_time_ns
        _CACHE["last_res"] = res
    return out


# revision 17
# speedup vs baseline: 1.0141x; 1.0141x over previous
"""BatchRGATLayer Trainium2 kernel (8 NeuronCores, data-parallel over (batch, row-half)).

kernel(**inputs) takes FULL inputs (x, edge, adj, W, W1, a), shards across 8
cores (core c -> batch c//2, rows (c%2)*256 .. +256), runs one SPMD Bass
program on all 8 cores, gathers to the full (4, 512, 256) output.

For row-half cores (c%2==1) the node axis is rolled by -256 on the host for
x, edge(j), adj(j) so the single SPMD program can treat local rows as [0,256).
Softmax and att@h are invariant to a consistent j-permutation.

Device algorithm per core (rows R=256 of one batch):
  h = x @ W (PE);  s_i = h_local @ a1, s_j = h @ a2 (DVE/PE)
  s_e[i,j] = sum_e edge[i,j,e] * (W1@a3)[e]  -- dominant stream, on DVE:
    edge is DMA'd with inline fp32->fp16 cast (SWDGE), multiplied by the
    broadcast w-vector in 2x mode, then segment-summed by a 3-level
    fp16 pairwise-add tree + small reduce (tensor_reduce is 1x-only).
  softmax over j without max-subtraction (logits are small; exp gets a
  -11 bias to stay in fp16 range; adj<=0 handled by multiplicative mask).
  h' = att @ h via PE (fp16), out = elu(h'/denom) = max(exp(min(x,0))-1, x).

Emission order keeps the DVE queue free of setup dependencies so the edge
stream starts as soon as the first edge tile lands.
"""

import sys

sys.path.insert(0, "/opt/trn_rl_repo")

from contextlib import ExitStack

import numpy as np

import concourse.bass as bass
import concourse.tile as tile
from concourse import bacc, mybir
from concourse.bass_utils import run_bass_kernel_spmd
from concourse.masks import make_identity

F32 = mybir.dt.float32
F16 = mybir.dt.float16
AF = mybir.ActivationFunctionType
ALU = mybir.AluOpType

# problem dims (hardcoded per spec)
B, N, IN_F, E_F, OUT_F = 4, 512, 256, 64, 256
R = 256
N_CORES = 8
ALPHA = 0.2
EXP_BIAS = -11.0

# tunables
JB = 128         # j-block per edge DMA tile: [128, JB*64] fp16
ED_BUFS = 4
PROD_BUFS = 2
DEBUG_NJB = None
DEBUG_NIT = None

_CACHE = {}


def build_program():
    nc = bacc.Bacc("TRN2", target_bir_lowering=False, debug=False)

    edge_d = nc.dram_tensor("edge_s", [R, N, E_F], F32, kind="ExternalInput").ap()
    adj_d = nc.dram_tensor("adj_s", [R, N], F32, kind="ExternalInput").ap()
    x_d = nc.dram_tensor("x_b", [N, IN_F], F32, kind="ExternalInput").ap()
    w_d = nc.dram_tensor("W", [IN_F, OUT_F], F32, kind="ExternalInput").ap()
    w1_d = nc.dram_tensor("W1", [E_F, OUT_F], F32, kind="ExternalInput").ap()
    a_d = nc.dram_tensor("a", [3 * OUT_F, 1], F32, kind="ExternalInput").ap()
    out_d = nc.dram_tensor("out_s", [R, OUT_F], F32, kind="ExternalOutput").ap()

    NIT = R // 128
    NJT = N // 128
    NFT = IN_F // 128
    NOT_ = OUT_F // 128
    NJB = N // JB if DEBUG_NJB is None else DEBUG_NJB
    NIT_RUN = NIT if DEBUG_NIT is None else DEBUG_NIT

    ctx = ExitStack()
    with tile.TileContext(nc) as tc, ctx:
        consts = ctx.enter_context(tc.tile_pool(name="consts", bufs=1))
        sb1 = ctx.enter_context(tc.tile_pool(name="sb1", bufs=1))
        psx = ctx.enter_context(tc.tile_pool(name="psx", bufs=2, space="PSUM"))
        ed_pool = ctx.enter_context(tc.tile_pool(name="ed", bufs=ED_BUFS))
        prod_pool = ctx.enter_context(tc.tile_pool(name="prod", bufs=PROD_BUFS))
        soft_pool = ctx.enter_context(tc.tile_pool(name="soft", bufs=2))
        att_ps_pool = ctx.enter_context(tc.tile_pool(name="att_ps", bufs=2, space="PSUM"))
        attT_pool = ctx.enter_context(tc.tile_pool(name="attT", bufs=2))
        hp_ps_pool = ctx.enter_context(tc.tile_pool(name="hp_ps", bufs=2, space="PSUM"))
        out_pool = ctx.enter_context(tc.tile_pool(name="outp", bufs=2))

        # ---- persistent tiles ----
        ident = consts.tile([128, 128], F32)
        ident16 = consts.tile([128, 128], F16)
        ones_row = consts.tile([1, 128], F32)
        expbias = consts.tile([128, 1], F32)
        w_all = consts.tile([128, NFT * OUT_F], F32)
        w_sb = [w_all[:, bass.ts(ft, OUT_F)] for ft in range(NFT)]
        w1_sb = consts.tile([E_F, OUT_F], F32)
        a_all = consts.tile([1, 3 * OUT_F], F32)
        a1_row = a_all[:, 0:OUT_F]
        a3_row = a_all[:, 2 * OUT_F : 3 * OUT_F]
        acol6 = consts.tile([128, 6], F32)
        a2_col = [acol6[:, 2 + ot : 3 + ot] for ot in range(NOT_)]
        x_all = sb1.tile([128, NJT * IN_F], F32)
        x_sb = [x_all[:, bass.ts(rt, IN_F)] for rt in range(NJT)]
        adj_all = sb1.tile([128, NIT * N], F32)
        adj_sb = [adj_all[:, bass.ts(it, N)] for it in range(NIT)]
        xT_sb = [sb1.tile([128, N], F32, tag=f"xT{ft}", name=f"xT{ft}") for ft in range(NFT)]
        h_sb = [sb1.tile([128, OUT_F], F32, tag=f"h{rt}", name=f"h{rt}") for rt in range(NJT)]
        h16_sb = [sb1.tile([128, OUT_F], F16, tag=f"h16_{rt}", name=f"h16_{rt}") for rt in range(NJT)]
        hT_sb = [sb1.tile([128, N], F32, tag=f"hT{ot}", name=f"hT{ot}") for ot in range(NOT_)]
        mask_sb = [sb1.tile([128, N], F16, tag=f"mk{it}", name=f"mk{it}") for it in range(NIT)]
        si_col = [sb1.tile([128, 1], F32, tag=f"si{it}", name=f"si{it}") for it in range(NIT)]
        se_dve = [sb1.tile([128, N], F32, tag=f"se{it}", name=f"se{it}") for it in range(NIT)]
        sj_rep = sb1.tile([128, N], F32)
        wcol = consts.tile([128, E_F], F16)
        w2 = consts.tile([128, 2], F32)
        w1a3 = consts.tile([E_F, 1], F32)
        w1a3_row = consts.tile([1, E_F], F32)

        # ---- input DMAs (few, coalesced; issued before edge flood) ----
        nc.sync.dma_start(a_all[:], a_d[:, :].rearrange("a b -> b a"))
        nc.sync.dma_start(w1_sb[:], w1_d[:, :])
        nc.sync.dma_start(
            acol6[:].rearrange("p (c b) -> p c b", b=1),
            a_d[:, :].rearrange("(c p) b -> p c b", p=128),
        )
        nc.sync.dma_start(
            w_all[:].rearrange("p (ft f) -> p ft f", f=OUT_F),
            w_d[:, :].rearrange("(ft p) f -> p ft f", p=128),
        )
        nc.sync.dma_start(
            x_all[:].rearrange("p (rt f) -> p rt f", f=IN_F),
            x_d[:, :].rearrange("(rt p) f -> p rt f", p=128),
        )
        nc.sync.dma_start(
            adj_all[:].rearrange("p (it j) -> p it j", j=N),
            adj_d[:, :].rearrange("(it p) j -> p it j", p=128),
        )

        # ---- setup part 1: constants the edge stream needs (wcol) ----
        nc.gpsimd.memset(ones_row[:], 1.0)
        nc.gpsimd.memset(expbias[:], EXP_BIAS)
        make_identity(nc, ident)
        make_identity(nc, ident16)

        a3_rep_ps = psx.tile([E_F, OUT_F], F32, tag="mps")
        nc.tensor.matmul(a3_rep_ps[:], ones_row[:, 0:E_F], a3_row[:])
        a3_rep = sb1.tile([E_F, OUT_F], F32)
        nc.scalar.copy(a3_rep[:], a3_rep_ps[:])
        ttr_scratch = sb1.tile([E_F, OUT_F], F32)
        nc.vector.tensor_tensor(ttr_scratch[:], w1_sb[:], a3_rep[:], ALU.mult)
        nc.vector.reduce_sum(w1a3[:], ttr_scratch[:], axis=mybir.AxisListType.X)
        w1a3_row_ps = psx.tile([1, E_F], F32, tag="mps")
        nc.tensor.transpose(w1a3_row_ps[:], w1a3[:], ident[0:E_F, 0:E_F])
        nc.scalar.copy(w1a3_row[:], w1a3_row_ps[:])
        wcol_ps = psx.tile([128, E_F], F32, tag="mps")
        nc.tensor.matmul(wcol_ps[:], ones_row[:], w1a3_row[:])
        nc.scalar.copy(wcol[:], wcol_ps[:])
        nc.gpsimd.memset(w2[:], 0.0)
        nc.vector.tensor_copy(w2[0:E_F, 0:1], w1a3[:])
        nc.sync.dma_start(w2[E_F:128, 1:2], w1a3[:])

        def setup2():
            # xT via PE transposes
            for rt in range(NJT):
                xt_ps = psx.tile([128, NFT * 128], F32, tag="mps")
                for ft in range(NFT):
                    nc.tensor.transpose(
                        xt_ps[:, bass.ts(ft, 128)], x_sb[rt][:, bass.ts(ft, 128)], ident[:]
                    )
                for ft in range(NFT):
                    nc.scalar.copy(xT_sb[ft][:, bass.ts(rt, 128)], xt_ps[:, bass.ts(ft, 128)])
            # h = x @ W
            for rt in range(NJT):
                h_ps = psx.tile([128, OUT_F], F32, tag="mps")
                for ft in range(NFT):
                    nc.tensor.matmul(
                        h_ps[:], xT_sb[ft][:, bass.ts(rt, 128)], w_sb[ft][:],
                        start=(ft == 0), stop=(ft == NFT - 1),
                    )
                nc.scalar.copy(h_sb[rt][:], h_ps[:])
            for rt in range(NJT):
                nc.vector.tensor_copy(h16_sb[rt][:], h_sb[rt][:])
            # hT = W^T x^T
            for ot in range(NOT_):
                for rt in range(NJT):
                    ht_ps = psx.tile([128, 128], F32, tag="mps")
                    for ft in range(NFT):
                        nc.tensor.matmul(
                            ht_ps[:],
                            w_sb[ft][:, bass.ts(ot, 128)],
                            xT_sb[ft][:, bass.ts(rt, 128)],
                            start=(ft == 0), stop=(ft == NFT - 1),
                        )
                    nc.scalar.copy(hT_sb[ot][:, bass.ts(rt, 128)], ht_ps[:])
            # s_i for local rows (rows [0, R) of h)
            a1rep_ps = psx.tile([128, OUT_F], F32, tag="mps")
            nc.tensor.matmul(a1rep_ps[:], ones_row[:], a1_row[:])
            a1rep = sb1.tile([128, OUT_F], F32)
            nc.scalar.copy(a1rep[:], a1rep_ps[:])
            for it in range(NIT):
                scratch = sb1.tile([128, OUT_F], F32, tag="sisc")
                nc.vector.tensor_tensor(scratch[:], h_sb[it][:], a1rep[:], ALU.mult)
                nc.vector.reduce_sum(si_col[it][:], scratch[:], axis=mybir.AxisListType.X)
            # s_j for all nodes, replicated across partitions
            sj_ps = psx.tile([1, N], F32, tag="mps")
            for ot in range(NOT_):
                nc.tensor.matmul(
                    sj_ps[:], a2_col[ot][:], hT_sb[ot][:],
                    start=(ot == 0), stop=(ot == NOT_ - 1),
                )
            sj_row = sb1.tile([1, N], F32)
            nc.scalar.copy(sj_row[:], sj_ps[:])
            sjrep_ps = psx.tile([128, N], F32, tag="mps")
            nc.tensor.matmul(sjrep_ps[:], ones_row[:], sj_row[:])
            nc.scalar.copy(sj_rep[:], sjrep_ps[:])
            # masks (1.0 where adj > 0)
            for it in range(NIT):
                nc.vector.tensor_scalar(
                    mask_sb[it][:], adj_sb[it][:], 0.0, None, op0=ALU.is_gt
                )

        for it in range(NIT_RUN):
            # ---- edge stream: s_e via fp16 mult + pairwise tree ----
            for jb in range(NJB):
                ed = ed_pool.tile([128, JB * E_F], F16)
                nc.gpsimd.dma_start(
                    ed[:],
                    edge_d[bass.ts(it, 128), bass.ts(jb, JB), :].rearrange(
                        "p a b -> p (a b)"
                    ),
                )
                prod = prod_pool.tile([128, JB * E_F], F16)
                nc.vector.tensor_tensor(
                    prod[:].rearrange("p (a b) -> p a b", b=E_F),
                    ed[:].rearrange("p (a b) -> p a b", b=E_F),
                    wcol[:, None, :].broadcast_to([128, JB, E_F]),
                    ALU.mult,
                )
                v0 = prod[:].rearrange("p (a b) -> p a b", b=E_F)
                t1 = prod_pool.tile([128, JB * 32], F16, tag="t1")
                nc.vector.tensor_tensor(
                    t1[:].rearrange("p (a b) -> p a b", b=32),
                    v0[:, :, 0:32], v0[:, :, 32:64], ALU.add,
                )
                v1 = t1[:].rearrange("p (a b) -> p a b", b=32)
                t2 = prod_pool.tile([128, JB * 16], F16, tag="t2")
                nc.vector.tensor_tensor(
                    t2[:].rearrange("p (a b) -> p a b", b=16),
                    v1[:, :, 0:16], v1[:, :, 16:32], ALU.add,
                )
                v2 = t2[:].rearrange("p (a b) -> p a b", b=16)
                t3 = prod_pool.tile([128, JB * 8], F16, tag="t3")
                nc.vector.tensor_tensor(
                    t3[:].rearrange("p (a b) -> p a b", b=8),
                    v2[:, :, 0:8], v2[:, :, 8:16], ALU.add,
                )
                nc.vector.reduce_sum(
                    se_dve[it][:, bass.ts(jb, JB)],
                    t3[:].rearrange("p (a b) -> p a b", b=8),
                    axis=mybir.AxisListType.X,
                )

            if it == 0:
                setup2()

            # ---- softmax (no max-subtraction; exp biased into fp16 range) ----
            z = soft_pool.tile([128, N], F32, tag="z")
            nc.vector.scalar_tensor_tensor(
                out=z[:], in0=se_dve[it][:], scalar=si_col[it][:],
                in1=sj_rep[:], op0=ALU.add, op1=ALU.add,
            )
            zl = soft_pool.tile([128, N], F32, tag="zl")
            nc.vector.scalar_tensor_tensor(
                out=zl[:], in0=z[:], scalar=ALPHA, in1=z[:], op0=ALU.mult, op1=ALU.max
            )
            p = soft_pool.tile([128, N], F16, tag="p")
            nc.scalar.activation(p[:], zl[:], AF.Exp, bias=expbias[:])
            pm = soft_pool.tile([128, N], F16, tag="pm")
            nc.vector.tensor_tensor(pm[:], p[:], mask_sb[it][:], ALU.mult)
            denom = soft_pool.tile([128, 1], F32, tag="den")
            nc.vector.reduce_sum(denom[:], pm[:], axis=mybir.AxisListType.X)
            rden = soft_pool.tile([128, 1], F32, tag="rden")
            nc.vector.reciprocal(rden[:], denom[:])

            # ---- h' = att @ h (fp16 PE path) ----
            hp_ps = hp_ps_pool.tile([128, OUT_F], F32)
            for jt in range(NJT):
                aps = att_ps_pool.tile([128, 128], F16)
                nc.tensor.transpose(aps[:], pm[:, bass.ts(jt, 128)], ident16[:])
                asb = attT_pool.tile([128, 128], F16)
                nc.scalar.copy(asb[:], aps[:])
                nc.tensor.matmul(
                    hp_ps[:], asb[:], h16_sb[jt][:],
                    start=(jt == 0), stop=(jt == NJT - 1),
                )

            # ---- normalize + ELU + store ----
            xx = out_pool.tile([128, OUT_F], F32, tag="xx")
            nc.scalar.mul(xx[:], hp_ps[:], rden[:])
            tmin = out_pool.tile([128, OUT_F], F32, tag="tm")
            nc.vector.tensor_scalar(tmin[:], xx[:], 0.0, None, op0=ALU.min)
            ex = out_pool.tile([128, OUT_F], F32, tag="ex")
            nc.scalar.activation(ex[:], tmin[:], AF.Exp)
            ot_sb = out_pool.tile([128, OUT_F], F32, tag="ot")
            nc.vector.scalar_tensor_tensor(
                out=ot_sb[:], in0=ex[:], scalar=-1.0, in1=xx[:], op0=ALU.add, op1=ALU.max
            )
            nc.sync.dma_start(out_d[bass.ts(it, 128), :], ot_sb[:])

        for it_ in range(NIT_RUN, NIT):
            o = sb1.tile([128, OUT_F], F32, tag=f"pad{it_}", name=f"pad{it_}")
            nc.gpsimd.memset(o[:], 0.0)
            nc.sync.dma_start(out_d[bass.ts(it_, 128), :], o[:])

    nc.compile()
    return nc


def _shard(x, edge, adj, W, W1, a):
    in_maps = []
    for c in range(N_CORES):
        bi, half = c // 2, c % 2
        r0 = half * R
        if r0:
            xb = np.roll(x[bi], -r0, axis=0)
            ed = np.roll(edge[bi, r0 : r0 + R], -r0, axis=1)
            ad = np.roll(adj[bi, r0 : r0 + R], -r0, axis=1)
        else:
            xb = x[bi]
            ed = edge[bi, 0:R]
            ad = adj[bi, 0:R]
        in_maps.append(
            {
                "edge_s": np.ascontiguousarray(ed),
                "adj_s": np.ascontiguousarray(ad),
                "x_b": np.ascontiguousarray(xb),
                "W": W,
                "W1": W1,
                "a": a,
            }
        )
    return in_maps


def kernel(x, edge, adj, W, W1, a, _trace=False):
    if "nc" not in _CACHE:
        _CACHE["nc"] = build_program()
    nc = _CACHE["nc"]

    x = np.asarray(x, dtype=np.float32)
    edge = np.asarray(edge, dtype=np.float32)
    adj = np.asarray(adj, dtype=np.float32)
    W = np.ascontiguousarray(np.asarray(W, dtype=np.float32))
    W1 = np.ascontiguousarray(np.asarray(W1, dtype=np.float32))
    a = np.ascontiguousarray(np.asarray(a, dtype=np.float32).reshape(3 * OUT_F, 1))

    in_maps = _shard(x, edge, adj, W, W1, a)
    res = run_bass_kernel_spmd(nc, in_maps, core_ids=list(range(N_CORES)), trace=_trace)
    out = np.empty((B, N, OUT_F), dtype=np.float32)
    for c in range(N_CORES):
        bi, half = c // 2, c % 2
        out[bi, half * R : (half + 1) * R] = res.results[c]["out_s"]
    if _trace:
        _CACHE["last_exec_time_ns"] = res.exec_time_ns
        _CACHE["last_res"] = res
    return out
